# revision 1
# baseline (speedup 1.0000x reference)
"""MoE adapter (router + rank-16 expert adapters) Trainium2 Bass kernel.

Problem: x[8,4096,1024] f32; router Linear(1024->8), softmax, top-2 (renormalized);
per-expert adapter down(1024->16), relu, up(16->1024) + bias, weighted-summed
by the dense top-2 gate weights.

Math identity used: with w[t,e] the dense (zero for non-top2) normalized gates,
  out[t,:] = sum_e w[t,e] * (relu(x@Wd_e + bd_e) @ Wu_e + bu_e)
           = (w_expand ⊙ relu(x@WdFlat + bdFlat)) @ WuFlat + w @ bu
where WdFlat:[1024,128], WuFlat:[128,1024] stack experts (er = 16e+r), and
w_expand[t,16e+r] = w[t,e]. The softmax denominator cancels in top-2
renormalization, so only exp(logit - rowmax) is needed.

Sharding: pure data parallel. Tokens (B*S = 32768) split 8 ways; core i takes
x[i] (= batch row i). Weights replicated. No collectives.

Per-core pipeline (stripe = 512 tokens, 8 stripes):
  1. DMA x stripe in token-major [128t, 1024k] (4 blocks).
  2. PE-transpose into X^T [128k, 512t] per k-chunk (8 chunks).
  3. Router matmul (fp32r, weights stationary, tokens streaming N=512) ->
     logits^T [8e, 512t]; PE-transpose small blocks back to token-major.
  4. Top-2 gate math on VectorE/ScalarE -> w [128t, 8e]; PE-transpose to
     w^T[8,512]; broadcast to er rows via SEL matmul -> w_bcast [128er, 512t].
  5. Down matmul (fp32r) -> H^T [128er, 512t]; ScalarE fuses bias+relu during
     PSUM evacuation; VectorE multiplies by w_bcast -> H'^T.
  6. Up matmul with H'^T block as stationary (fp32r) naturally yields
     token-major out [128t, 1024d]; a K=8 matmul with w^T stationary
     accumulates the w@bu bias into the same PSUM. Evacuate + DMA out.
"""

import sys

sys.path.insert(0, "/opt/trn_rl_repo")

from contextlib import ExitStack

import numpy as np

import concourse.bacc as bacc
import concourse.bass as bass
import concourse.mybir as mybir
import concourse.tile as tile

F32 = mybir.dt.float32
F32R = mybir.dt.float32r

B, S, D = 8, 4096, 1024
E, R, TOP_K = 8, 16, 2
ER = E * R  # 128
N_CORES = 8
T_CORE = B * S // N_CORES  # 4096 tokens per core
STRIPE = 512
NBLK = STRIPE // 128  # 4
KC = D // 128  # 8 k-chunks


def _build_program(t_core: int = T_CORE, fast_math: bool = True, time_loops: int = 1, cfg: dict | None = None):
    """Build the Bass program for one core processing t_core tokens.

    time_loops > 1 wraps the whole token loop in a hardware For_i that
    recomputes the same output N times — used only for timing (slope method
    cancels dispatch overhead)."""
    nc = bacc.Bacc("TRN2", target_bir_lowering=False, debug=False)
    cfg = dict(cfg or {})
    CF = {
        "xin_bufs": 8, "xtp_bufs": 2, "hs_bufs": 2, "hp_bufs": 2, "smal_bufs": 2,
        "outp_bufs": 4, "pt_bufs": 3, "psm_bufs": 2, "ph_bufs": 1, "pwb_bufs": 1,
        "po_bufs": 1, "xh_eng": "scalar", "xl_eng": "vector", "out_eng": "scalar",
    }
    CF.update(cfg)

    x = nc.dram_tensor("x", [t_core, D], F32, kind="ExternalInput").ap()
    wds = nc.dram_tensor("wds", [128, D], F32R, kind="ExternalInput").ap()
    wrs = nc.dram_tensor("wrs", [128, KC * E], F32, kind="ExternalInput").ap()
    wus = nc.dram_tensor("wus", [ER, D], F32R, kind="ExternalInput").ap()
    bus = nc.dram_tensor("bus", [E, D], F32R, kind="ExternalInput").ap()
    bds = nc.dram_tensor("bds", [128, 1], F32, kind="ExternalInput").ap()
    brb = nc.dram_tensor("brb", [128, NBLK * E], F32, kind="ExternalInput").ap()
    i128 = nc.dram_tensor("i128", [128, 128], F32, kind="ExternalInput").ap()
    i128r = nc.dram_tensor("i128r", [128, 128], F32R, kind="ExternalInput").ap()
    sel = nc.dram_tensor("sel", [E, ER], F32R, kind="ExternalInput").ap()
    wz = nc.dram_tensor("wz", [128, KC * 40], F32R, kind="ExternalInput").ap()
    out = nc.dram_tensor("out", [t_core, D], F32, kind="ExternalOutput").ap()

    n_stripes = t_core // STRIPE
    assert t_core % STRIPE == 0

    with tile.TileContext(nc) as tc, ExitStack() as ctx:
        const = ctx.enter_context(tc.tile_pool(name="const", bufs=1))
        xin = ctx.enter_context(tc.tile_pool(name="xin", bufs=CF["xin_bufs"]))
        xtp = ctx.enter_context(tc.tile_pool(name="xt", bufs=CF["xtp_bufs"]))
        hsp = ctx.enter_context(tc.tile_pool(name="hs", bufs=CF["hs_bufs"]))
        hpp = ctx.enter_context(tc.tile_pool(name="hp", bufs=CF["hp_bufs"]))
        smal = ctx.enter_context(tc.tile_pool(name="smal", bufs=CF["smal_bufs"]))
        outp = ctx.enter_context(tc.tile_pool(name="outsb", bufs=CF["outp_bufs"]))
        ptp = ctx.enter_context(tc.tile_pool(name="pt", bufs=CF["pt_bufs"], space="PSUM"))
        psm = ctx.enter_context(tc.tile_pool(name="psmall", bufs=CF["psm_bufs"], space="PSUM"))
        php = ctx.enter_context(tc.tile_pool(name="ph", bufs=CF["ph_bufs"], space="PSUM"))
        pwb = ctx.enter_context(tc.tile_pool(name="pwb", bufs=CF["pwb_bufs"], space="PSUM"))
        pop = ctx.enter_context(tc.tile_pool(name="po", bufs=CF["po_bufs"], space="PSUM"))
        wbp = ctx.enter_context(tc.tile_pool(name="wb_sb", bufs=2))

        # --- load identity + first-stripe x before the bulk weight loads so
        # the PE can start transposing immediately ---
        i128_t = const.tile([128, 128], F32)
        nc.sync.dma_start(i128_t[:], i128)
        pre_x = []
        if CF.get("hoist_x0", True):
            for b in range(NBLK):
                xb = xin.tile([128, D], F32, tag="xin")
                nc.sync.dma_start(xb[:], x[b * 128 : (b + 1) * 128, :])
                pre_x.append(xb)

        # --- load weights / constants (once) ---
        wds_t = const.tile([128, D], F32R)
        nc.sync.dma_start(wds_t[:], wds)
        wrsf_t = const.tile([128, KC * E], F32)
        nc.sync.dma_start(wrsf_t[:], wrs)
        wus_t = const.tile([ER, D], F32R)
        nc.sync.dma_start(wus_t[:], wus)
        bus_t = const.tile([E, D], F32R)
        nc.sync.dma_start(bus_t[:], bus)
        bds_t = const.tile([128, 1], F32)
        nc.sync.dma_start(bds_t[:], bds)
        brb_t = const.tile([128, NBLK * E], F32)
        nc.sync.dma_start(brb_t[:], brb)
        i128r_t = const.tile([128, 128], F32R)
        nc.sync.dma_start(i128r_t[:], i128r)
        # split router weights into [hi | lo] pairs per k-chunk (one-time).
        # 40-wide stationary per chunk: hi logits land on PSUM partitions 0:8,
        # lo on 32:40 (transpose lhsT base_partition must be 0/32/64).
        MW = 40
        wrs_hl = const.tile([128, KC * MW], F32R)
        nc.sync.dma_start(wrs_hl[:], wz)
        for c in range(KC):
            hi = wrs_hl[:, c * MW : c * MW + 8]
            lo = wrs_hl[:, c * MW + 32 : c * MW + 40]
            nc.vector.tensor_copy(hi, wrsf_t[:, c * E : (c + 1) * E])
            nc.vector.tensor_sub(lo, wrsf_t[:, c * E : (c + 1) * E], hi)
        sel_t = const.tile([E, ER], F32R)
        nc.sync.dma_start(sel_t[:], sel)

        def stripe_body(s):
            tok0 = s * STRIPE
            # --- 1. load x stripe, token-major ---
            if s == 0 and pre_x:
                xts = pre_x
            else:
                xts = []
                for b in range(NBLK):
                    xb = xin.tile([128, D], F32, tag="xin")
                    nc.sync.dma_start(
                        xb[:], x[tok0 + b * 128 : tok0 + (b + 1) * 128, :]
                    )
                    xts.append(xb)

            # --- 2. PE-transpose to X^T chunks ---
            xt_all = xtp.tile([128, KC * STRIPE], F32R)
            xl_all = xtp.tile([128, KC * STRIPE], F32R, tag="xl_all")
            for c in range(KC):
                pt = ptp.tile([128, STRIPE], F32, tag="pt")
                for b in range(NBLK):
                    nc.tensor.transpose(
                        pt[:, b * 128 : (b + 1) * 128],
                        xts[b][:, c * 128 : (c + 1) * 128],
                        i128_t[:],
                    )
                dst = xt_all[:, c * STRIPE : (c + 1) * STRIPE]
                dstl = xl_all[:, c * STRIPE : (c + 1) * STRIPE]
                if CF["xh_eng"] == "scalar":
                    nc.scalar.copy(dst, pt[:])
                else:
                    nc.vector.tensor_copy(dst, pt[:])
                nc.vector.tensor_sub(dstl, pt[:], dst)

            def xtc(c):
                return xt_all[:, c * STRIPE : (c + 1) * STRIPE]

            def xlc(c):
                return xl_all[:, c * STRIPE : (c + 1) * STRIPE]

            # --- 3. router logits^T = Wr^T X ---
            plg = psm.tile([MW, STRIPE], F32, tag="psmall")
            for c in range(KC):
                nc.tensor.matmul(
                    plg[:],
                    wrs_hl[:, c * MW : (c + 1) * MW],
                    xtc(c),
                    start=(c == 0),
                    stop=False,
                )
            for c in range(KC):
                nc.tensor.matmul(
                    plg[:],
                    wrs_hl[:, c * MW : (c + 1) * MW],
                    xlc(c),
                    start=False,
                    stop=(c == KC - 1),
                )
            lgt = smal.tile([MW, STRIPE], F32R, tag="lgt")
            nc.vector.tensor_copy(lgt[:], plg[:])
            # transpose hi and lo row groups into separate token-major PSUM
            # tiles; they become partition-aligned so DVE can add them
            plgtm = psm.tile([128, NBLK * E], F32R, tag="psmall")
            plgtm_lo = psm.tile([128, NBLK * E], F32R, tag="psmall")
            for b in range(NBLK):
                nc.tensor.transpose(
                    plgtm[:, b * E : (b + 1) * E],
                    lgt[0:E, b * 128 : (b + 1) * 128],
                    i128r_t[:E, :E],
                )
                nc.tensor.transpose(
                    plgtm_lo[:, b * E : (b + 1) * E],
                    lgt[32 : 32 + E, b * 128 : (b + 1) * 128],
                    i128r_t[32 : 32 + E, 32 : 32 + E],
                )

            # --- 4. top-2 gate math (token-major, 4 blocks side by side) ---
            def v3(ap):
                return ap.rearrange("p (b e) -> p b e", e=E)

            lg2 = smal.tile([128, NBLK * E], F32, tag="lg2")
            nc.vector.tensor_add(lg2[:], plgtm[:], brb_t[:])
            nc.vector.tensor_add(lg2[:], lg2[:], plgtm_lo[:])
            rmax = smal.tile([128, NBLK], F32, tag="rmax")
            nc.vector.reduce_max(out=rmax[:], in_=v3(lg2[:]), axis=mybir.AxisListType.X)
            nrmax = smal.tile([128, NBLK], F32, tag="nrmax")
            nc.vector.tensor_scalar_mul(nrmax[:], rmax[:], -1.0)
            ex = smal.tile([128, NBLK * E], F32, tag="ex")
            for b in range(NBLK):
                nc.scalar.activation(
                    ex[:, b * E : (b + 1) * E],
                    lg2[:, b * E : (b + 1) * E],
                    mybir.ActivationFunctionType.Exp,
                    bias=nrmax[:, b : b + 1],
                )
            m1 = smal.tile([128, NBLK], F32, tag="m1")
            nc.vector.reduce_max(out=m1[:], in_=v3(ex[:]), axis=mybir.AxisListType.X)
            exm = smal.tile([128, NBLK * E], F32, tag="exm")
            for b in range(NBLK):
                nc.vector.tensor_scalar(
                    out=exm[:, b * E : (b + 1) * E],
                    in0=ex[:, b * E : (b + 1) * E],
                    scalar1=m1[:, b : b + 1],
                    scalar2=None,
                    op0=mybir.AluOpType.is_lt,
                )
            nc.vector.tensor_mul(exm[:], exm[:], ex[:])
            m2 = smal.tile([128, NBLK], F32, tag="m2")
            nc.vector.reduce_max(out=m2[:], in_=v3(exm[:]), axis=mybir.AxisListType.X)
            kp = smal.tile([128, NBLK * E], F32, tag="kp")
            for b in range(NBLK):
                nc.vector.tensor_scalar(
                    out=kp[:, b * E : (b + 1) * E],
                    in0=ex[:, b * E : (b + 1) * E],
                    scalar1=m2[:, b : b + 1],
                    scalar2=None,
                    op0=mybir.AluOpType.is_ge,
                )
            nc.vector.tensor_mul(kp[:], kp[:], ex[:])
            den = smal.tile([128, NBLK], F32, tag="den")
            nc.vector.reduce_sum(out=den[:], in_=v3(kp[:]), axis=mybir.AxisListType.X)
            dinv = smal.tile([128, NBLK], F32, tag="dinv")
            nc.vector.reciprocal(dinv[:], den[:])
            w = smal.tile([128, NBLK * E], F32R, tag="w")
            for b in range(NBLK):
                nc.vector.tensor_scalar_mul(
                    w[:, b * E : (b + 1) * E],
                    kp[:, b * E : (b + 1) * E],
                    dinv[:, b : b + 1],
                )

            # --- w^T [8, 512] and w_bcast [128er, 512t] ---
            pwt = psm.tile([E, STRIPE], F32R, tag="psmall")
            for b in range(NBLK):
                nc.tensor.transpose(
                    pwt[:, b * 128 : (b + 1) * 128],
                    w[:, b * E : (b + 1) * E],
                    i128r_t[:],
                )
            wt = smal.tile([E, STRIPE], F32R, tag="wt")
            nc.scalar.copy(wt[:], pwt[:])
            if CF.get("wb_dma", False):
                pb = wbp.tile([128, STRIPE], F32R, tag="wb_sb")
                src_ap = wt[:].unsqueeze(1).broadcast_to([E, R, STRIPE])
                nc.gpsimd.dma_start(pb[:].rearrange("(e r) t -> e r t", r=R), src_ap)
            else:
                pb = pwb.tile([128, STRIPE], F32)
                nc.tensor.matmul(pb[:], sel_t[:], wt[:], start=True, stop=True)

            # --- 5. down matmul -> H^T; relu+bias on evac; gate-scale ---
            ph = php.tile([128, STRIPE], F32)
            for c in range(KC):
                nc.tensor.matmul(
                    ph[:],
                    wds_t[:, c * 128 : (c + 1) * 128],
                    xtc(c),
                    start=(c == 0),
                    stop=(c == KC - 1),
                )
            hs = hsp.tile([128, STRIPE], F32)
            nc.scalar.activation(
                hs[:], ph[:], mybir.ActivationFunctionType.Relu, bias=bds_t[:, 0:1]
            )
            hp = hpp.tile([128, STRIPE], F32R)
            nc.vector.tensor_mul(hp[:], hs[:], pb[:])

            # --- 6. up + bu-bias, token-major out, store ---
            for b in range(NBLK):
                osb = outp.tile([128, D], F32, tag="osb")
                for h2 in range(2):
                    po = pop.tile([128, 512], F32, tag="po")
                    nc.tensor.matmul(
                        po[:],
                        hp[:, b * 128 : (b + 1) * 128],
                        wus_t[:, h2 * 512 : (h2 + 1) * 512],
                        start=True,
                        stop=False,
                        skip_group_check=True,
                    )
                    nc.tensor.matmul(
                        po[:],
                        wt[:, b * 128 : (b + 1) * 128],
                        bus_t[:, h2 * 512 : (h2 + 1) * 512],
                        start=False,
                        stop=True,
                        skip_group_check=True,
                    )
                    dst = osb[:, h2 * 512 : (h2 + 1) * 512]
                    oe = CF["out_eng"]
                    use_v = (b + h2) % 2 == 0 if oe == "alt" else (oe == "vector")
                    if use_v:
                        nc.vector.tensor_copy(dst, po[:])
                    else:
                        nc.scalar.copy(dst, po[:])
                nc.sync.dma_start(
                    out[tok0 + b * 128 : tok0 + (b + 1) * 128, :], osb[:]
                )

        if time_loops > 1:
            with tc.For_i(0, time_loops, 1):
                for s in range(n_stripes):
                    stripe_body(s)
        else:
            for s in range(n_stripes):
                stripe_body(s)
    nc.compile()
    return nc


def _prep_weights(Wr, br, Wd, bd, Wu, bu):
    """Host-side weight layout preprocessing (all tiny)."""
    Wr = np.asarray(Wr, np.float32)
    br = np.asarray(br, np.float32)
    Wd = np.asarray(Wd, np.float32)
    bd = np.asarray(bd, np.float32)
    Wu = np.asarray(Wu, np.float32)
    bu = np.asarray(bu, np.float32)
    # wds[p, c*128 + e*16 + r] = Wd[e, c*128+p, r]
    wds = np.ascontiguousarray(
        Wd.reshape(E, KC, 128, R).transpose(2, 1, 0, 3).reshape(128, KC * E * R)
    )
    # order check: index (p, c, e, r) -> col c*(E*R) + e*R + r. We want chunk-
    # major columns [c][er]: exactly that.
    # wrs[p, c*E + e] = Wr[c*128+p, e]
    wrs = np.ascontiguousarray(
        Wr.reshape(KC, 128, E).transpose(1, 0, 2).reshape(128, KC * E)
    )
    i128r = np.eye(128, dtype=np.float32)
    wus = np.ascontiguousarray(Wu.reshape(ER, D))
    bus = np.ascontiguousarray(bu)
    bds = np.ascontiguousarray(bd.reshape(ER, 1))
    brb = np.ascontiguousarray(np.tile(br, (128, NBLK)))
    i128 = np.eye(128, dtype=np.float32)
    sel_m = np.zeros((E, ER), np.float32)
    for e in range(E):
        sel_m[e, e * R : (e + 1) * R] = 1.0
    return dict(wds=wds, wrs=wrs, wus=wus, bus=bus, bds=bds, brb=brb, i128=i128, i128r=i128r, sel=sel_m, wz=np.zeros((128, KC * 40), np.float32))


_NC_CACHE = {}


def _get_program(t_core=T_CORE, fast_math=True):
    key = (t_core, fast_math)
    if key not in _NC_CACHE:
        _NC_CACHE[key] = _build_program(t_core, fast_math)
    return _NC_CACHE[key]


def kernel(x, Wr, br, Wd, bd, Wu, bu):
    from concourse.bass_utils import run_bass_kernel_spmd

    x = np.asarray(x, np.float32)
    wmap = _prep_weights(Wr, br, Wd, bd, Wu, bu)
    xf = np.ascontiguousarray(x.reshape(B * S, D))
    nc = _get_program()
    in_maps = []
    for i in range(N_CORES):
        m = dict(wmap)
        m["x"] = xf[i * T_CORE : (i + 1) * T_CORE]
        in_maps.append(m)
    res = run_bass_kernel_spmd(nc, in_maps, list(range(N_CORES)))
    outs = [res.results[i]["out"] for i in range(N_CORES)]
    return np.concatenate(outs, axis=0).reshape(B, S, D)



# revision 8
# speedup vs baseline: 1.4636x; 1.4636x over previous
"""MoE adapter (router + rank-16 expert adapters) Trainium2 Bass kernel.

Problem: x[8,4096,1024] f32; router Linear(1024->8), softmax, top-2 (renormalized);
per-expert adapter down(1024->16), relu, up(16->1024) + bias, weighted-summed
by the dense top-2 gate weights.

Math identity used: with w[t,e] the dense (zero for non-top2) normalized gates,
  out[t,:] = sum_e w[t,e] * (relu(x@Wd_e + bd_e) @ Wu_e + bu_e)
           = (w_expand ⊙ relu(x@WdFlat + bdFlat)) @ WuFlat + w @ bu
where WdFlat:[1024,128], WuFlat:[128,1024] stack experts (er = 16e+r), and
w_expand[t,16e+r] = w[t,e]. The softmax denominator cancels in top-2
renormalization, so only exp(logit - rowmax) is needed.

Sharding: pure data parallel. Tokens (B*S = 32768) split 8 ways; core i takes
x[i] (= batch row i). Weights replicated. No collectives.

Per-core pipeline (stripe = 512 tokens, 8 stripes):
  1. DMA x stripe in token-major [128t, 1024k] (4 blocks).
  2. PE-transpose into X^T [128k, 512t] per k-chunk (8 chunks).
  3. Router matmul (fp32r, weights stationary, tokens streaming N=512) ->
     logits^T [8e, 512t]; PE-transpose small blocks back to token-major.
  4. Top-2 gate math on VectorE/ScalarE -> w [128t, 8e]; PE-transpose to
     w^T[8,512]; broadcast to er rows via SEL matmul -> w_bcast [128er, 512t].
  5. Down matmul (fp32r) -> H^T [128er, 512t]; ScalarE fuses bias+relu during
     PSUM evacuation; VectorE multiplies by w_bcast -> H'^T.
  6. Up matmul with H'^T block as stationary (fp32r) naturally yields
     token-major out [128t, 1024d]; a K=8 matmul with w^T stationary
     accumulates the w@bu bias into the same PSUM. Evacuate + DMA out.
"""

import sys

sys.path.insert(0, "/opt/trn_rl_repo")

from contextlib import ExitStack

import numpy as np

import concourse.bacc as bacc
import concourse.bass as bass
import concourse.mybir as mybir
import concourse.tile as tile

F32 = mybir.dt.float32
F32R = mybir.dt.float32r

B, S, D = 8, 4096, 1024
E, R, TOP_K = 8, 16, 2
ER = E * R  # 128
N_CORES = 8
T_CORE = B * S // N_CORES  # 4096 tokens per core
STRIPE = 512
NBLK = STRIPE // 128  # 4
KC = D // 128  # 8 k-chunks


def _build_program(t_core: int = T_CORE, fast_math: bool = True, time_loops: int = 1, cfg: dict | None = None):
    """Build the Bass program for one core processing t_core tokens.

    time_loops > 1 wraps the whole token loop in a hardware For_i that
    recomputes the same output N times — used only for timing (slope method
    cancels dispatch overhead)."""
    nc = bacc.Bacc("TRN2", target_bir_lowering=False, debug=False)
    cfg = dict(cfg or {})
    CF = {
        "xin_bufs": 8, "xtp_bufs": 2, "hs_bufs": 2, "hp_bufs": 2, "smal_bufs": 2,
        "outp_bufs": 4, "pt_bufs": 3, "psm_bufs": 2, "ph_bufs": 1, "pwb_bufs": 1,
        "po_bufs": 1, "xh_eng": "scalar", "xl_eng": "vector", "out_eng": "scalar",
    }
    CF.update(cfg)

    x = nc.dram_tensor(
        "x", [t_core, D], F32R if cfg.get("tp_f32r", False) else F32, kind="ExternalInput"
    ).ap()
    wds = nc.dram_tensor("wds", [128, D], F32R, kind="ExternalInput").ap()
    wrs = nc.dram_tensor(
        "wrs",
        [128, KC * E],
        F32R if cfg.get("simple_router", False) else F32,
        kind="ExternalInput",
    ).ap()
    wus = nc.dram_tensor("wus", [ER, D], F32R, kind="ExternalInput").ap()
    bus = nc.dram_tensor("bus", [E, D], F32R, kind="ExternalInput").ap()
    bds = nc.dram_tensor("bds", [128, 1], F32, kind="ExternalInput").ap()
    brb = nc.dram_tensor("brb", [128, NBLK * E], F32, kind="ExternalInput").ap()
    i128 = nc.dram_tensor("i128", [128, 128], F32, kind="ExternalInput").ap()
    i128r = nc.dram_tensor("i128r", [128, 128], F32R, kind="ExternalInput").ap()
    sel = nc.dram_tensor("sel", [E, ER], F32R, kind="ExternalInput").ap()
    wz = nc.dram_tensor("wz", [128, KC * 40], F32R, kind="ExternalInput").ap()
    out = nc.dram_tensor("out", [t_core, D], F32, kind="ExternalOutput").ap()

    n_stripes = t_core // STRIPE
    assert t_core % STRIPE == 0

    with tile.TileContext(nc) as tc, ExitStack() as ctx:
        const = ctx.enter_context(tc.tile_pool(name="const", bufs=1))
        xin = ctx.enter_context(tc.tile_pool(name="xin", bufs=CF["xin_bufs"]))
        xtp = ctx.enter_context(tc.tile_pool(name="xt", bufs=CF["xtp_bufs"]))
        hsp = ctx.enter_context(tc.tile_pool(name="hs", bufs=CF["hs_bufs"]))
        hpp = ctx.enter_context(tc.tile_pool(name="hp", bufs=CF["hp_bufs"]))
        smal = ctx.enter_context(tc.tile_pool(name="smal", bufs=CF["smal_bufs"]))
        outp = ctx.enter_context(tc.tile_pool(name="outsb", bufs=CF["outp_bufs"]))
        ptp = ctx.enter_context(tc.tile_pool(name="pt", bufs=CF["pt_bufs"], space="PSUM"))
        psm = ctx.enter_context(tc.tile_pool(name="psmall", bufs=CF["psm_bufs"], space="PSUM"))
        php = ctx.enter_context(tc.tile_pool(name="ph", bufs=CF["ph_bufs"], space="PSUM"))
        pwb = ctx.enter_context(tc.tile_pool(name="pwb", bufs=CF["pwb_bufs"], space="PSUM"))
        pop = ctx.enter_context(tc.tile_pool(name="po", bufs=CF["po_bufs"], space="PSUM"))
        wbp = ctx.enter_context(tc.tile_pool(name="wb_sb", bufs=2))

        simple_router = CF.get("simple_router", False)
        tp_f32r = CF.get("tp_f32r", False)
        XIN_DT = F32R if tp_f32r else F32

        # --- load identity + first-stripe x before the bulk weight loads so
        # the PE can start transposing immediately ---
        i128_t = const.tile([128, 128], F32)
        nc.sync.dma_start(i128_t[:], i128)
        pre_x = []
        if CF.get("hoist_x0", True):
            for b in range(NBLK):
                xb = xin.tile([128, D], XIN_DT, tag="xin")
                nc.sync.dma_start(xb[:], x[b * 128 : (b + 1) * 128, :])
                pre_x.append(xb)

        # --- load weights / constants (once) ---
        wds_t = const.tile([128, D], F32R)
        nc.sync.dma_start(wds_t[:], wds)
        wrsf_t = const.tile([128, KC * E], F32R if CF.get("simple_router", False) else F32)
        nc.sync.dma_start(wrsf_t[:], wrs)
        wus_t = const.tile([ER, D], F32R)
        nc.sync.dma_start(wus_t[:], wus)
        bus_t = const.tile([E, D], F32R)
        nc.sync.dma_start(bus_t[:], bus)
        bds_t = const.tile([128, 1], F32)
        nc.sync.dma_start(bds_t[:], bds)
        brb_t = const.tile([128, NBLK * E], F32)
        nc.sync.dma_start(brb_t[:], brb)
        i128r_t = const.tile([128, 128], F32R)
        nc.sync.dma_start(i128r_t[:], i128r)
        # split router weights into [hi | lo] pairs per k-chunk (one-time).
        # 40-wide stationary per chunk: hi logits land on PSUM partitions 0:8,
        # lo on 32:40 (transpose lhsT base_partition must be 0/32/64).
        MW = 40
        if not simple_router:
            wrs_hl = const.tile([128, KC * MW], F32R)
            nc.sync.dma_start(wrs_hl[:], wz)
            for c in range(KC):
                hi = wrs_hl[:, c * MW : c * MW + 8]
                lo = wrs_hl[:, c * MW + 32 : c * MW + 40]
                nc.vector.tensor_copy(hi, wrsf_t[:, c * E : (c + 1) * E])
                nc.vector.tensor_sub(lo, wrsf_t[:, c * E : (c + 1) * E], hi)
        sel_t = const.tile([E, ER], F32R)
        nc.sync.dma_start(sel_t[:], sel)

        def stripe_body(s):
            tok0 = s * STRIPE
            # --- 1. load x stripe, token-major ---
            if s == 0 and pre_x:
                xts = pre_x
            else:
                xts = []
                for b in range(NBLK):
                    xb = xin.tile([128, D], XIN_DT, tag="xin")
                    nc.sync.dma_start(
                        xb[:], x[tok0 + b * 128 : tok0 + (b + 1) * 128, :]
                    )
                    xts.append(xb)

            # --- 2. PE-transpose to X^T chunks ---
            xt_all = xtp.tile([128, KC * STRIPE], F32R)
            if not simple_router:
                xl_all = xtp.tile([128, KC * STRIPE], F32R, tag="xl_all")
            for c in range(KC):
                pt = ptp.tile([128, STRIPE], XIN_DT, tag="pt")
                for b in range(NBLK):
                    nc.tensor.transpose(
                        pt[:, b * 128 : (b + 1) * 128],
                        xts[b][:, c * 128 : (c + 1) * 128],
                        i128r_t[:] if tp_f32r else i128_t[:],
                    )
                dst = xt_all[:, c * STRIPE : (c + 1) * STRIPE]
                if CF["xh_eng"] == "scalar":
                    nc.scalar.copy(dst, pt[:])
                else:
                    nc.vector.tensor_copy(dst, pt[:])
                if not simple_router:
                    dstl = xl_all[:, c * STRIPE : (c + 1) * STRIPE]
                    nc.vector.tensor_sub(dstl, pt[:], dst)

            def xtc(c):
                return xt_all[:, c * STRIPE : (c + 1) * STRIPE]

            def xlc(c):
                return xl_all[:, c * STRIPE : (c + 1) * STRIPE]

            # --- 3. router logits^T = Wr^T X ---
            if simple_router:
                plg = psm.tile([E, STRIPE], F32, tag="psmall")
                for c in range(KC):
                    nc.tensor.matmul(
                        plg[:],
                        wrsf_t[:, c * E : (c + 1) * E],
                        xtc(c),
                        start=(c == 0),
                        stop=(c == KC - 1),
                    )
                lgt = smal.tile([E, STRIPE], F32R, tag="lgt")
                nc.vector.tensor_copy(lgt[:], plg[:])
                plgtm = psm.tile([128, NBLK * E], F32R, tag="psmall")
                for b in range(NBLK):
                    nc.tensor.transpose(
                        plgtm[:, b * E : (b + 1) * E],
                        lgt[0:E, b * 128 : (b + 1) * 128],
                        i128r_t[:E, :E],
                    )
            else:
                plg = psm.tile([MW, STRIPE], F32, tag="psmall")
                for c in range(KC):
                    nc.tensor.matmul(
                        plg[:],
                        wrs_hl[:, c * MW : (c + 1) * MW],
                        xtc(c),
                        start=(c == 0),
                        stop=False,
                    )
                for c in range(KC):
                    nc.tensor.matmul(
                        plg[:],
                        wrs_hl[:, c * MW : (c + 1) * MW],
                        xlc(c),
                        start=False,
                        stop=(c == KC - 1),
                    )
                lgt = smal.tile([MW, STRIPE], F32R, tag="lgt")
                nc.vector.tensor_copy(lgt[:], plg[:])
                # transpose hi and lo row groups into separate token-major PSUM
                # tiles; they become partition-aligned so DVE can add them
                plgtm = psm.tile([128, NBLK * E], F32R, tag="psmall")
                plgtm_lo = psm.tile([128, NBLK * E], F32R, tag="psmall")
                for b in range(NBLK):
                    nc.tensor.transpose(
                        plgtm[:, b * E : (b + 1) * E],
                        lgt[0:E, b * 128 : (b + 1) * 128],
                        i128r_t[:E, :E],
                    )
                    nc.tensor.transpose(
                        plgtm_lo[:, b * E : (b + 1) * E],
                        lgt[32 : 32 + E, b * 128 : (b + 1) * 128],
                        i128r_t[32 : 32 + E, 32 : 32 + E],
                    )

            # --- 4. top-2 gate math (token-major, 4 blocks side by side) ---
            def v3(ap):
                return ap.rearrange("p (b e) -> p b e", e=E)

            lg2 = smal.tile([128, NBLK * E], F32, tag="lg2")
            nc.vector.tensor_add(lg2[:], plgtm[:], brb_t[:])
            if not simple_router:
                nc.vector.tensor_add(lg2[:], lg2[:], plgtm_lo[:])
            rmax = smal.tile([128, NBLK], F32, tag="rmax")
            nc.vector.reduce_max(out=rmax[:], in_=v3(lg2[:]), axis=mybir.AxisListType.X)
            nrmax = smal.tile([128, NBLK], F32, tag="nrmax")
            nc.vector.tensor_scalar_mul(nrmax[:], rmax[:], -1.0)
            ex = smal.tile([128, NBLK * E], F32, tag="ex")
            for b in range(NBLK):
                nc.scalar.activation(
                    ex[:, b * E : (b + 1) * E],
                    lg2[:, b * E : (b + 1) * E],
                    mybir.ActivationFunctionType.Exp,
                    bias=nrmax[:, b : b + 1],
                )
            m1 = smal.tile([128, NBLK], F32, tag="m1")
            nc.vector.reduce_max(out=m1[:], in_=v3(ex[:]), axis=mybir.AxisListType.X)
            exm = smal.tile([128, NBLK * E], F32, tag="exm")
            for b in range(NBLK):
                nc.vector.tensor_scalar(
                    out=exm[:, b * E : (b + 1) * E],
                    in0=ex[:, b * E : (b + 1) * E],
                    scalar1=m1[:, b : b + 1],
                    scalar2=None,
                    op0=mybir.AluOpType.is_lt,
                )
            nc.vector.tensor_mul(exm[:], exm[:], ex[:])
            m2 = smal.tile([128, NBLK], F32, tag="m2")
            nc.vector.reduce_max(out=m2[:], in_=v3(exm[:]), axis=mybir.AxisListType.X)
            kp = smal.tile([128, NBLK * E], F32, tag="kp")
            for b in range(NBLK):
                nc.vector.tensor_scalar(
                    out=kp[:, b * E : (b + 1) * E],
                    in0=ex[:, b * E : (b + 1) * E],
                    scalar1=m2[:, b : b + 1],
                    scalar2=None,
                    op0=mybir.AluOpType.is_ge,
                )
            nc.vector.tensor_mul(kp[:], kp[:], ex[:])
            den = smal.tile([128, NBLK], F32, tag="den")
            nc.vector.reduce_sum(out=den[:], in_=v3(kp[:]), axis=mybir.AxisListType.X)
            dinv = smal.tile([128, NBLK], F32, tag="dinv")
            nc.vector.reciprocal(dinv[:], den[:])
            w = smal.tile([128, NBLK * E], F32R, tag="w")
            for b in range(NBLK):
                nc.vector.tensor_scalar_mul(
                    w[:, b * E : (b + 1) * E],
                    kp[:, b * E : (b + 1) * E],
                    dinv[:, b : b + 1],
                )

            # --- w^T [8, 512] and w_bcast [128er, 512t] ---
            pwt = psm.tile([E, STRIPE], F32R, tag="psmall")
            for b in range(NBLK):
                nc.tensor.transpose(
                    pwt[:, b * 128 : (b + 1) * 128],
                    w[:, b * E : (b + 1) * E],
                    i128r_t[:],
                )
            wt = smal.tile([E, STRIPE], F32R, tag="wt")
            nc.scalar.copy(wt[:], pwt[:])
            if CF.get("wb_dma", False):
                pb = wbp.tile([128, STRIPE], F32R, tag="wb_sb")
                src_ap = wt[:].unsqueeze(1).broadcast_to([E, R, STRIPE])
                nc.gpsimd.dma_start(pb[:].rearrange("(e r) t -> e r t", r=R), src_ap)
            else:
                pb = pwb.tile([128, STRIPE], F32)
                nc.tensor.matmul(pb[:], sel_t[:], wt[:], start=True, stop=True)

            # --- 5. down matmul -> H^T; relu+bias on evac; gate-scale ---
            ph = php.tile([128, STRIPE], F32)
            for c in range(KC):
                nc.tensor.matmul(
                    ph[:],
                    wds_t[:, c * 128 : (c + 1) * 128],
                    xtc(c),
                    start=(c == 0),
                    stop=(c == KC - 1),
                )
            hs = hsp.tile([128, STRIPE], F32)
            nc.scalar.activation(
                hs[:], ph[:], mybir.ActivationFunctionType.Relu, bias=bds_t[:, 0:1]
            )
            hp = hpp.tile([128, STRIPE], F32R)
            nc.vector.tensor_mul(hp[:], hs[:], pb[:])

            # --- 6. up + bu-bias, token-major out, store ---
            for b in range(NBLK):
                osb = outp.tile([128, D], F32, tag="osb")
                for h2 in range(2):
                    po = pop.tile([128, 512], F32, tag="po")
                    nc.tensor.matmul(
                        po[:],
                        hp[:, b * 128 : (b + 1) * 128],
                        wus_t[:, h2 * 512 : (h2 + 1) * 512],
                        start=True,
                        stop=False,
                        skip_group_check=True,
                    )
                    nc.tensor.matmul(
                        po[:],
                        wt[:, b * 128 : (b + 1) * 128],
                        bus_t[:, h2 * 512 : (h2 + 1) * 512],
                        start=False,
                        stop=True,
                        skip_group_check=True,
                    )
                    dst = osb[:, h2 * 512 : (h2 + 1) * 512]
                    oe = CF["out_eng"]
                    use_v = (b + h2) % 2 == 0 if oe == "alt" else (oe == "vector")
                    if use_v:
                        nc.vector.tensor_copy(dst, po[:])
                    else:
                        nc.scalar.copy(dst, po[:])
                nc.sync.dma_start(
                    out[tok0 + b * 128 : tok0 + (b + 1) * 128, :], osb[:]
                )

        if time_loops > 1:
            with tc.For_i(0, time_loops, 1):
                for s in range(n_stripes):
                    stripe_body(s)
        else:
            for s in range(n_stripes):
                stripe_body(s)
    nc.compile()
    return nc


def _prep_weights(Wr, br, Wd, bd, Wu, bu):
    """Host-side weight layout preprocessing (all tiny)."""
    Wr = np.asarray(Wr, np.float32)
    br = np.asarray(br, np.float32)
    Wd = np.asarray(Wd, np.float32)
    bd = np.asarray(bd, np.float32)
    Wu = np.asarray(Wu, np.float32)
    bu = np.asarray(bu, np.float32)
    # wds[p, c*128 + e*16 + r] = Wd[e, c*128+p, r]
    wds = np.ascontiguousarray(
        Wd.reshape(E, KC, 128, R).transpose(2, 1, 0, 3).reshape(128, KC * E * R)
    )
    # order check: index (p, c, e, r) -> col c*(E*R) + e*R + r. We want chunk-
    # major columns [c][er]: exactly that.
    # wrs[p, c*E + e] = Wr[c*128+p, e]
    wrs = np.ascontiguousarray(
        Wr.reshape(KC, 128, E).transpose(1, 0, 2).reshape(128, KC * E)
    )
    i128r = np.eye(128, dtype=np.float32)
    wus = np.ascontiguousarray(Wu.reshape(ER, D))
    bus = np.ascontiguousarray(bu)
    bds = np.ascontiguousarray(bd.reshape(ER, 1))
    brb = np.ascontiguousarray(np.tile(br, (128, NBLK)))
    i128 = np.eye(128, dtype=np.float32)
    sel_m = np.zeros((E, ER), np.float32)
    for e in range(E):
        sel_m[e, e * R : (e + 1) * R] = 1.0
    return dict(wds=wds, wrs=wrs, wus=wus, bus=bus, bds=bds, brb=brb, i128=i128, i128r=i128r, sel=sel_m, wz=np.zeros((128, KC * 40), np.float32))


_NC_CACHE = {}


def _get_program(t_core=T_CORE, fast_math=True):
    key = (t_core, fast_math)
    if key not in _NC_CACHE:
        _NC_CACHE[key] = _build_program(t_core, fast_math)
    return _NC_CACHE[key]


def kernel(x, Wr, br, Wd, bd, Wu, bu):
    from concourse.bass_utils import run_bass_kernel_spmd

    x = np.asarray(x, np.float32)
    wmap = _prep_weights(Wr, br, Wd, bd, Wu, bu)
    xf = np.ascontiguousarray(x.reshape(B * S, D))
    nc = _get_program()
    in_maps = []
    for i in range(N_CORES):
        m = dict(wmap)
        m["x"] = xf[i * T_CORE : (i + 1) * T_CORE]
        in_maps.append(m)
    res = run_bass_kernel_spmd(nc, in_maps, list(range(N_CORES)))
    outs = [res.results[i]["out"] for i in range(N_CORES)]
    return np.concatenate(outs, axis=0).reshape(B, S, D)

